# revision 5
# baseline (speedup 1.0000x reference)
"""Trainium2 Bass kernel for a 2-layer GRU (S=512, B=64, H=IN=1024).

Strategy: 8-way tensor-parallel over the hidden dimension. Core c owns rows
[128c, 128c+128) of the hidden state for both layers. Per layer:
  - input projections gi = x @ Wi^T (+bias) as a big token-chunked GEMM,
    each core computing its 3x128 rows of the 3H gate dimension,
  - the 512-step recurrence with Wh slices stationary in SBUF (bf16),
    h^T moving; after each step the 8 cores AllGather the new hidden
    state (bf16) so everyone has the full h^T for the next step.
Matmuls run in bf16 with fp32 PSUM accumulation; the per-core hidden state
is kept in fp32 and only the matmul operands are rounded.
"""

import sys

sys.path.insert(0, "/opt/trn_rl_repo")

import ml_dtypes
import numpy as np

import concourse.bacc as bacc
import concourse.bass as bass
import concourse.mybir as mybir
import concourse.tile as tile
from concourse.bass_utils import run_bass_kernel_spmd

BF16 = mybir.dt.bfloat16
F32 = mybir.dt.float32
AF = mybir.ActivationFunctionType
ALU = mybir.AluOpType

B = 64
H = 1024
L = 2
NC = 8
KT = H // 128  # contraction k-tiles
MR = 128  # hidden rows owned per core


def build_nc(S):
    NCH = (S * B) // 512  # 512-token chunks per layer for the gi GEMM
    nc = bacc.Bacc("TRN2", target_bir_lowering=False, debug=False, num_devices=NC)

    # ---- I/O ----
    xT = nc.dram_tensor("xT", [H, S * B], BF16, kind="ExternalInput")
    wi_in = nc.dram_tensor("wi_sb", [128, L * 3 * KT * 128], BF16, kind="ExternalInput")
    wh_in = nc.dram_tensor("wh_sb", [128, L * 3 * KT * 128], BF16, kind="ExternalInput")
    bias_in = nc.dram_tensor("bias_sb", [128, 8], F32, kind="ExternalInput")
    h0_own_in = nc.dram_tensor("h0_own", [L * 128, B], F32, kind="ExternalInput")
    h0_all_in = nc.dram_tensor("h0_all", [L * 128, KT * B], BF16, kind="ExternalInput")
    out_seq = nc.dram_tensor("out_seq", [S * 128, B], F32, kind="ExternalOutput")
    out_hn = nc.dram_tensor("out_hn", [L * 128, B], F32, kind="ExternalOutput")

    rg = [list(range(NC))]

    with tile.TileContext(nc) as tc:
        with (
            tc.tile_pool(name="wpool", bufs=1) as wpool,
            tc.tile_pool(name="xchunk", bufs=2) as xchunk_pool,
            tc.tile_pool(name="giout", bufs=3) as giout_pool,
            tc.tile_pool(name="gpsum", bufs=3, space="PSUM") as gpsum_pool,
            tc.tile_pool(name="rzpsum", bufs=2, space="PSUM") as rzpsum_pool,
            tc.tile_pool(name="npsum", bufs=2, space="PSUM") as npsum_pool,
            tc.tile_pool(name="hpool", bufs=2) as hpool,
            tc.tile_pool(name="gtile", bufs=3) as gtile_pool,
            tc.tile_pool(name="ew", bufs=3) as ew_pool,
            tc.tile_pool(name="hf", bufs=3) as hf_pool,
            tc.tile_pool(name="hb", bufs=3) as hb_pool,
            tc.tile_pool(name="dram", bufs=1, space="DRAM") as dram_pool,
            tc.tile_pool(name="agin", bufs=3, space="DRAM") as agin_pool,
            tc.tile_pool(name="agout", bufs=3, space="DRAM") as agout_pool,
        ):
            # ---- persistent SBUF ----
            WI = wpool.tile([128, L * 3 * KT * 128], BF16, tag="wi")
            WH = wpool.tile([128, L * 3 * KT * 128], BF16, tag="wh")
            BIAS = wpool.tile([128, 8], F32, tag="bias")
            nc.sync.dma_start(out=WI[:], in_=wi_in[:])
            nc.sync.dma_start(out=WH[:], in_=wh_in[:])
            nc.sync.dma_start(out=BIAS[:], in_=bias_in[:])

            # ---- DRAM intermediates ----
            gi_dram = [
                dram_pool.tile([S * 128, 192], BF16, tag=f"gi{l}", name=f"gi{l}")
                for l in range(L)
            ]
            y1T = dram_pool.tile([S * 1024, B], BF16, tag="y1T", name="y1T")

            def wslice(wt, l, g, k):
                m = (l * 3 + g) * KT + k
                return wt[:, m * 128 : (m + 1) * 128]

            def gi_gemm(l, rhs_dram, rhs_is_xT):
                """gi[l] = (input @ Wi[l]^T + bias) for this core's 3x128 rows.

                rhs_dram: [H, S*B] (xT layout) or [S*1024, B] (y1T layout).
                """
                for c in range(NCH):
                    xt = xchunk_pool.tile([128, KT * 512], BF16, tag="xc")
                    if rhs_is_xT:
                        nc.sync.dma_start(
                            out=xt[:].rearrange("p (k n) -> p k n", k=KT),
                            in_=rhs_dram[:, c * 512 : (c + 1) * 512].rearrange(
                                "(k p) n -> p k n", p=128
                            ),
                        )
                    else:
                        # y1T rows are [t, k, p]-major: t*1024 + k*128 + p.
                        # DMA APs are limited to 3 dims, so one DMA per k-tile.
                        chunk = rhs_dram[c * 8 * 1024 : (c + 1) * 8 * 1024, :]
                        for k in range(KT):
                            nc.sync.dma_start(
                                out=xt[:, k * 512 : (k + 1) * 512].rearrange(
                                    "p (t b) -> p t b", t=8
                                ),
                                in_=chunk.rearrange(
                                    "(t k p) b -> k p t b", k=KT, p=128
                                )[k],
                            )
                    for g in range(3):
                        ps = gpsum_pool.tile([128, 512], F32, tag="gp")
                        for k in range(KT):
                            nc.tensor.matmul(
                                ps[:],
                                lhsT=wslice(WI, l, g, k),
                                rhs=xt[:, k * 512 : (k + 1) * 512],
                                start=(k == 0),
                                stop=(k == KT - 1),
                            )
                        ot = giout_pool.tile([128, 512], BF16, tag="go")
                        nc.vector.tensor_scalar_add(
                            ot[:], ps[:], BIAS[:, l * 3 + g : l * 3 + g + 1]
                        )
                        # 512 cols = 8 steps x 64 batch -> gi rows
                        nc.sync.dma_start(
                            out=gi_dram[l][
                                c * 8 * 128 : (c + 1) * 8 * 128,
                                g * 64 : (g + 1) * 64,
                            ].rearrange("(t p) b -> p t b", p=128),
                            in_=ot[:].rearrange("p (t b) -> p t b", t=8),
                        )

            def recurrence(l):
                HF = hf_pool.tile([128, B], F32, tag="hf")
                nc.sync.dma_start(out=HF[:], in_=h0_own_in[l * 128 : (l + 1) * 128, :])
                Hcur = hpool.tile([128, KT * B], BF16, tag="h")
                nc.sync.dma_start(
                    out=Hcur[:], in_=h0_all_in[l * 128 : (l + 1) * 128, :]
                )
                for t in range(S):
                    G = gtile_pool.tile([128, 192], BF16, tag="g")
                    nc.sync.dma_start(
                        out=G[:], in_=gi_dram[l][t * 128 : (t + 1) * 128, :]
                    )
                    P_rz = rzpsum_pool.tile([128, 128], F32, tag="prz")
                    P_n = npsum_pool.tile([128, 64], F32, tag="pn")
                    for g in range(2):
                        for k in range(KT):
                            nc.tensor.matmul(
                                P_rz[:, g * 64 : (g + 1) * 64],
                                lhsT=wslice(WH, l, g, k),
                                rhs=Hcur[:, k * B : (k + 1) * B],
                                start=(k == 0),
                                stop=(k == KT - 1),
                            )
                    for k in range(KT):
                        nc.tensor.matmul(
                            P_n[:],
                            lhsT=wslice(WH, l, 2, k),
                            rhs=Hcur[:, k * B : (k + 1) * B],
                            start=(k == 0),
                            stop=(k == KT - 1),
                        )
                    # elementwise gates
                    A = ew_pool.tile([128, 128], F32, tag="a")
                    nc.vector.tensor_tensor(A[:], P_rz[:], G[:, 0:128], ALU.add)
                    RZ = ew_pool.tile([128, 128], F32, tag="rz")
                    nc.scalar.activation(RZ[:], A[:], AF.Sigmoid)
                    TN = ew_pool.tile([128, 64], F32, tag="tn")
                    # tn = (gh_n + bh_n) * r
                    nc.vector.scalar_tensor_tensor(
                        TN[:], P_n[:], BIAS[:, 6 + l : 7 + l], RZ[:, 0:64],
                        ALU.add, ALU.mult,
                    )
                    nc.vector.tensor_tensor(TN[:], TN[:], G[:, 128:192], ALU.add)
                    N = ew_pool.tile([128, 64], F32, tag="n")
                    nc.scalar.activation(N[:], TN[:], AF.Tanh)
                    # w = z*h ; m = n - z*n ; h' = m + w
                    W_ = ew_pool.tile([128, 64], F32, tag="w")
                    nc.vector.tensor_tensor(W_[:], RZ[:, 64:128], HF[:], ALU.mult)
                    M_ = ew_pool.tile([128, 64], F32, tag="m")
                    nc.vector.tensor_tensor(M_[:], RZ[:, 64:128], N[:], ALU.mult)
                    HFn = hf_pool.tile([128, B], F32, tag="hf")
                    nc.vector.tensor_tensor(HFn[:], N[:], M_[:], ALU.subtract)
                    nc.vector.tensor_tensor(HFn[:], HFn[:], W_[:], ALU.add)
                    HF = HFn
                    if l == L - 1:
                        nc.sync.dma_start(
                            out=out_seq[t * 128 : (t + 1) * 128, :], in_=HF[:]
                        )
                    last = t == S - 1
                    if l == L - 1 and last:
                        break  # no exchange needed after the final step
                    HB = hb_pool.tile([128, B], BF16, tag="hb")
                    nc.gpsimd.tensor_copy(HB[:], HF[:])
                    bin_ = agin_pool.tile([128, B], BF16, tag="bi")
                    nc.sync.dma_start(out=bin_[:], in_=HB[:])
                    bout = agout_pool.tile([1024, B], BF16, tag="bo")
                    nc.gpsimd.collective_compute(
                        "AllGather",
                        ALU.bypass,
                        replica_groups=rg,
                        ins=[bin_.opt()],
                        outs=[bout.opt()],
                    )
                    if l == 0:
                        nc.sync.dma_start(
                            out=y1T[t * 1024 : (t + 1) * 1024, :], in_=bout[:]
                        )
                    if not last:
                        Hn = hpool.tile([128, KT * B], BF16, tag="h")
                        nc.sync.dma_start(
                            out=Hn[:].rearrange("p (k b) -> p k b", k=KT),
                            in_=bout[:].rearrange("(k p) b -> p k b", p=128),
                        )
                        Hcur = Hn
                nc.sync.dma_start(out=out_hn[l * 128 : (l + 1) * 128, :], in_=HF[:])

            gi_gemm(0, xT, True)
            recurrence(0)
            gi_gemm(1, y1T, False)
            recurrence(1)

    nc.compile()
    return nc


def _pack_weights(W, c):
    # W: [L, 3, H, H] (out_h, in_h). Returns [128, L*3*KT*128] bf16: for
    # block m=(l*3+g)*KT+k, cols [128m,128m+128) hold W[l,g,128c:+128,128k:+128]^T.
    Wb = W.reshape(L, 3, NC, 128, KT, 128)[:, :, c]  # [L,3,128m,KT,128k]
    Wb = Wb.transpose(0, 1, 3, 4, 2).reshape(L * 3 * KT, 128, 128)
    return np.ascontiguousarray(
        Wb.transpose(1, 0, 2).reshape(128, L * 3 * KT * 128)
    ).astype(ml_dtypes.bfloat16)


def kernel(x, h_0, W_i, W_h, b_i, b_h):
    x = np.asarray(x, dtype=np.float32)
    h_0 = np.asarray(h_0, dtype=np.float32)
    W_i = np.asarray(W_i, dtype=np.float32)
    W_h = np.asarray(W_h, dtype=np.float32)
    b_i = np.asarray(b_i, dtype=np.float32)
    b_h = np.asarray(b_h, dtype=np.float32)
    S = x.shape[0]

    xT = np.ascontiguousarray(x.reshape(S * B, H).T).astype(ml_dtypes.bfloat16)
    # h0_all: [L*128, KT*B] bf16, same for all cores: full h^T rearranged
    h0T = h_0.transpose(0, 2, 1)  # [L, H, B]
    h0_all = np.ascontiguousarray(
        h0T.reshape(L, KT, 128, B).transpose(0, 2, 1, 3).reshape(L * 128, KT * B)
    ).astype(ml_dtypes.bfloat16)

    in_maps = []
    for c in range(NC):
        rows = slice(128 * c, 128 * (c + 1))
        bias_sb = np.zeros((128, 8), np.float32)
        for l in range(L):
            for g in range(3):
                fold = b_i[l, g, rows] + (b_h[l, g, rows] if g < 2 else 0.0)
                bias_sb[:, l * 3 + g] = fold
            bias_sb[:, 6 + l] = b_h[l, 2, rows]
        h0_own = np.ascontiguousarray(h0T[:, rows, :].reshape(L * 128, B)).astype(
            np.float32
        )
        in_maps.append(
            {
                "xT": xT,
                "wi_sb": _pack_weights(W_i, c),
                "wh_sb": _pack_weights(W_h, c),
                "bias_sb": bias_sb,
                "h0_own": h0_own,
                "h0_all": h0_all,
            }
        )

    nc = build_nc(S)
    res = run_bass_kernel_spmd(nc, in_maps, core_ids=list(range(NC)))

    seq = np.concatenate(
        [r["out_seq"].reshape(S, 128, B) for r in res.results], axis=1
    )  # [S, H, B]
    seq = np.ascontiguousarray(seq.transpose(0, 2, 1)).astype(np.float32)
    hn = np.concatenate(
        [r["out_hn"].reshape(L, 128, B) for r in res.results], axis=1
    )  # [L, H, B]
    hn = np.ascontiguousarray(hn.transpose(0, 2, 1)).astype(np.float32)
    return seq, hn


# revision 10
# speedup vs baseline: 1.4183x; 1.4183x over previous
"""Trainium2 Bass kernel for a 2-layer GRU (S=512, B=64, H=IN=1024).

Strategy: pure data-parallel over batch — core c owns batches [8c, 8c+8),
so the recurrence needs no inter-core communication (per-step collectives
measured ~100x their documented floor here). Each core holds the full
weight set (bf16) in SBUF and streams it through the PE array every step.

Layouts (per core, b = 8 local batches, j = 8 hidden k-tiles of 128):
  hidden state  HF:[128, 64] fp32, col = 8*j + b, partition p -> h row 128j+p
  gate preacts  gh^T in PSUM as [128, 64] per gate, same (j, b) cols
  matmul        out^T[(g,j) M-tiles, b] = Wh^T stationary, h^T moving (N=8)
Matmuls run in bf16 with fp32 PSUM accumulation; the recurrent state stays
fp32, only matmul operands are rounded to bf16.
"""

import sys

sys.path.insert(0, "/opt/trn_rl_repo")

import ml_dtypes
import numpy as np

import concourse.bacc as bacc
import concourse.bass as bass
import concourse.mybir as mybir
import concourse.tile as tile
from concourse.bass_utils import run_bass_kernel_spmd

BF16 = mybir.dt.bfloat16
F32 = mybir.dt.float32
AF = mybir.ActivationFunctionType
ALU = mybir.AluOpType

B = 64
H = 1024
L = 2
NC = 8
KT = H // 128  # contraction k-tiles
MT = 3 * KT  # M-tiles (gate-major: m = g*8 + j) per layer
BL = B // NC  # local batch


def build_nc(S, has_bias):
    TOK = S * BL  # tokens per core for the gi GEMMs
    CH = min(512, TOK)  # tokens per GEMM chunk
    CHS = CH // BL  # steps per chunk
    NCH = TOK // CH  # chunks
    nc = bacc.Bacc("TRN2", target_bir_lowering=False, debug=False, num_devices=NC)

    # ---- I/O ----
    xT = nc.dram_tensor("xT", [H, TOK], BF16, kind="ExternalInput")
    wi_in = nc.dram_tensor("wi_sb", [128, L * MT * KT * 128], BF16, kind="ExternalInput")
    wh_in = nc.dram_tensor("wh_sb", [128, L * MT * KT * 128], BF16, kind="ExternalInput")
    bias_in = nc.dram_tensor("bias_sb", [128, L * MT], F32, kind="ExternalInput")
    bhn_in = nc.dram_tensor("bhn_sb", [128, L * BL * KT], F32, kind="ExternalInput")
    h0_in = nc.dram_tensor("h0_own", [L * 128, BL * KT], F32, kind="ExternalInput")
    out_seq = nc.dram_tensor("out_seq", [S * 128, BL * KT], F32, kind="ExternalOutput")
    out_hn = nc.dram_tensor("out_hn", [L * 128, BL * KT], F32, kind="ExternalOutput")

    with tile.TileContext(nc) as tc:
        with (
            tc.tile_pool(name="wpool", bufs=1) as wpool,
            tc.tile_pool(name="xchunk", bufs=2) as xchunk_pool,
            tc.tile_pool(name="wistream", bufs=3) as wistream_pool,
            tc.tile_pool(name="giout", bufs=4) as giout_pool,
            tc.tile_pool(name="gpsum", bufs=4, space="PSUM") as gpsum_pool,
            tc.tile_pool(name="rzpsum", bufs=2, space="PSUM") as rzpsum_pool,
            tc.tile_pool(name="npsum", bufs=2, space="PSUM") as npsum_pool,
            tc.tile_pool(name="gtile", bufs=4) as gtile_pool,
            tc.tile_pool(name="ew", bufs=3) as ew_pool,
            tc.tile_pool(name="hf", bufs=3) as hf_pool,
            tc.tile_pool(name="hb", bufs=3) as hb_pool,
            tc.tile_pool(name="dram", bufs=1, space="DRAM") as dram_pool,
        ):
            # ---- persistent SBUF (Wi streams from DRAM per chunk) ----
            WH = wpool.tile([128, L * MT * KT * 128], BF16, tag="wh")
            BIAS = wpool.tile([128, L * MT], F32, tag="bias")
            nc.sync.dma_start(out=WH[:], in_=wh_in[:])
            nc.sync.dma_start(out=BIAS[:], in_=bias_in[:])
            if has_bias:
                BHN = wpool.tile([128, L * BL * KT], F32, tag="bhn")
                nc.sync.dma_start(out=BHN[:], in_=bhn_in[:])

            # ---- DRAM intermediates ----
            gi_dram = [
                dram_pool.tile([S * 128, 192], BF16, tag=f"gi{l}", name=f"gi{l}")
                for l in range(L)
            ]
            y1T = dram_pool.tile([S * 128, BL * KT], BF16, tag="y1T", name="y1T")

            def wslice(wt, l, m, k):
                i = (l * MT + m) * KT + k
                return wt[:, i * 128 : (i + 1) * 128]

            def gi_gemm(l, rhs_dram, rhs_is_xT):
                """gi[l]^T = Wi[l] @ input^T (+bias): all 3H rows, local tokens."""
                for c in range(NCH):
                    xt = xchunk_pool.tile([128, KT * CH], BF16, tag="xc")
                    if rhs_is_xT:
                        nc.sync.dma_start(
                            out=xt[:].rearrange("p (k n) -> p k n", k=KT),
                            in_=rhs_dram[:, c * CH : (c + 1) * CH].rearrange(
                                "(k p) n -> p k n", p=128
                            ),
                        )
                    else:
                        # y1T: rows (t, p), cols (j, b); k-tile j = cols 8j..
                        rows = rhs_dram[c * CHS * 128 : (c + 1) * CHS * 128, :]
                        for k in range(KT):
                            nc.sync.dma_start(
                                out=xt[:, k * CH : (k + 1) * CH].rearrange(
                                    "p (t b) -> p t b", t=CHS
                                ),
                                in_=rows[:, k * BL : (k + 1) * BL].rearrange(
                                    "(t p) b -> p t b", p=128
                                ),
                            )
                    for m in range(MT):
                        g, j = divmod(m, KT)
                        wt = wistream_pool.tile([128, KT * 128], BF16, tag="wt")
                        i0 = (l * MT + m) * KT * 128
                        nc.sync.dma_start(
                            out=wt[:], in_=wi_in[:, i0 : i0 + KT * 128]
                        )
                        ps = gpsum_pool.tile([128, CH], F32, tag="gp")
                        for k in range(KT):
                            nc.tensor.matmul(
                                ps[:],
                                lhsT=wt[:, k * 128 : (k + 1) * 128],
                                rhs=xt[:, k * CH : (k + 1) * CH],
                                start=(k == 0),
                                stop=(k == KT - 1),
                            )
                        ot = giout_pool.tile([128, CH], BF16, tag="go")
                        nc.vector.tensor_scalar_add(
                            ot[:], ps[:], BIAS[:, l * MT + m : l * MT + m + 1]
                        )
                        # CH cols = CHS steps x 8 batch -> gi rows (t,p)
                        nc.sync.dma_start(
                            out=gi_dram[l][
                                c * CHS * 128 : (c + 1) * CHS * 128,
                                g * 64 + j * BL : g * 64 + (j + 1) * BL,
                            ].rearrange("(t p) b -> p t b", p=128),
                            in_=ot[:].rearrange("p (t b) -> p t b", t=CHS),
                        )

            def recurrence(l):
                HF = hf_pool.tile([128, BL * KT], F32, tag="hf")
                nc.sync.dma_start(out=HF[:], in_=h0_in[l * 128 : (l + 1) * 128, :])
                HB = hb_pool.tile([128, BL * KT], BF16, tag="hb")
                nc.gpsimd.tensor_copy(HB[:], HF[:])
                for t in range(S):
                    G = gtile_pool.tile([128, 192], BF16, tag="g")
                    nc.sync.dma_start(
                        out=G[:], in_=gi_dram[l][t * 128 : (t + 1) * 128, :]
                    )
                    P_rz = rzpsum_pool.tile([128, 128], F32, tag="prz")
                    P_n = npsum_pool.tile([128, 64], F32, tag="pn")
                    for m in range(2 * KT):  # r and z gates
                        g, j = divmod(m, KT)
                        for k in range(KT):
                            nc.tensor.matmul(
                                P_rz[:, g * 64 + j * BL : g * 64 + (j + 1) * BL],
                                lhsT=wslice(WH, l, m, k),
                                rhs=HB[:, k * BL : (k + 1) * BL],
                                start=(k == 0),
                                stop=(k == KT - 1),
                            )
                    for j in range(KT):  # n gate
                        for k in range(KT):
                            nc.tensor.matmul(
                                P_n[:, j * BL : (j + 1) * BL],
                                lhsT=wslice(WH, l, 2 * KT + j, k),
                                rhs=HB[:, k * BL : (k + 1) * BL],
                                start=(k == 0),
                                stop=(k == KT - 1),
                            )
                    # elementwise gates
                    A = ew_pool.tile([128, 128], F32, tag="a")
                    nc.vector.tensor_tensor(A[:], P_rz[:], G[:, 0:128], ALU.add)
                    RZ = ew_pool.tile([128, 128], F32, tag="rz")
                    nc.scalar.activation(RZ[:], A[:], AF.Sigmoid)
                    TN = ew_pool.tile([128, 64], F32, tag="tn")
                    if has_bias:
                        nc.vector.tensor_tensor(
                            TN[:], P_n[:], BHN[:, l * 64 : (l + 1) * 64], ALU.add
                        )
                        nc.vector.tensor_tensor(TN[:], TN[:], RZ[:, 0:64], ALU.mult)
                    else:
                        nc.vector.tensor_tensor(TN[:], P_n[:], RZ[:, 0:64], ALU.mult)
                    nc.vector.tensor_tensor(TN[:], TN[:], G[:, 128:192], ALU.add)
                    N = ew_pool.tile([128, 64], F32, tag="n")
                    nc.scalar.activation(N[:], TN[:], AF.Tanh)
                    # h' = n - z*n + z*h
                    W_ = ew_pool.tile([128, 64], F32, tag="w")
                    nc.vector.tensor_tensor(W_[:], RZ[:, 64:128], HF[:], ALU.mult)
                    M_ = ew_pool.tile([128, 64], F32, tag="m")
                    nc.vector.tensor_tensor(M_[:], RZ[:, 64:128], N[:], ALU.mult)
                    HFn = hf_pool.tile([128, BL * KT], F32, tag="hf")
                    nc.vector.tensor_tensor(HFn[:], N[:], M_[:], ALU.subtract)
                    nc.vector.tensor_tensor(HFn[:], HFn[:], W_[:], ALU.add)
                    HF = HFn
                    HBn = hb_pool.tile([128, BL * KT], BF16, tag="hb")
                    nc.gpsimd.tensor_copy(HBn[:], HF[:])
                    HB = HBn
                    if l == 0:
                        nc.sync.dma_start(
                            out=y1T[t * 128 : (t + 1) * 128, :], in_=HB[:]
                        )
                    else:
                        nc.sync.dma_start(
                            out=out_seq[t * 128 : (t + 1) * 128, :], in_=HF[:]
                        )
                nc.sync.dma_start(out=out_hn[l * 128 : (l + 1) * 128, :], in_=HF[:])

            gi_gemm(0, xT, True)
            recurrence(0)
            gi_gemm(1, y1T, False)
            recurrence(1)

    nc.compile()
    return nc


def _pack_weights(W):
    # W: [L, 3, H, H] (out_h, in_h) -> [128, L*MT*KT*128] bf16.
    # Block i = (l*MT + g*KT + j)*KT + k holds W[l, g, 128j:+128, 128k:+128]^T.
    Wb = W.reshape(L, 3, KT, 128, KT, 128)  # [l, g, j, m, k, kk]
    Wb = Wb.transpose(0, 1, 2, 4, 5, 3).reshape(L * MT * KT, 128, 128)
    return np.ascontiguousarray(
        Wb.transpose(1, 0, 2).reshape(128, L * MT * KT * 128)
    ).astype(ml_dtypes.bfloat16)


def _jb_layout(a):
    # a: [lead, BL, H] -> [lead, 128, KT, BL] with col = j*BL + b
    lead = a.shape[0]
    a = a.reshape(lead, BL, KT, 128)
    return np.ascontiguousarray(a.transpose(0, 3, 2, 1))


def kernel(x, h_0, W_i, W_h, b_i, b_h):
    x = np.asarray(x, dtype=np.float32)
    h_0 = np.asarray(h_0, dtype=np.float32)
    W_i = np.asarray(W_i, dtype=np.float32)
    W_h = np.asarray(W_h, dtype=np.float32)
    b_i = np.asarray(b_i, dtype=np.float32)
    b_h = np.asarray(b_h, dtype=np.float32)
    S = x.shape[0]
    has_bias = bool(np.any(b_i) or np.any(b_h))

    wi_sb = _pack_weights(W_i)
    wh_sb = _pack_weights(W_h)
    # bias per (l, m=(g,j)): fold b_h into b_i for r,z gates (exact)
    bias_sb = np.zeros((128, L * MT), np.float32)
    for l in range(L):
        for g in range(3):
            for j in range(KT):
                v = b_i[l, g, 128 * j : 128 * (j + 1)].copy()
                if g < 2:
                    v += b_h[l, g, 128 * j : 128 * (j + 1)]
                bias_sb[:, l * MT + g * KT + j] = v
    # bh_n as a [128, (j,b)] tile per layer
    bhn_sb = np.zeros((128, L * BL * KT), np.float32)
    for l in range(L):
        v = b_h[l, 2].reshape(KT, 128)  # [j, p]
        bhn_sb[:, l * 64 : (l + 1) * 64] = np.repeat(
            v.T[:, :, None], BL, axis=2
        ).reshape(128, KT * BL)

    in_maps = []
    for c in range(NC):
        bs = slice(BL * c, BL * (c + 1))
        x_c = x[:, bs, :].reshape(S * BL, H)
        xT = np.ascontiguousarray(x_c.T).astype(ml_dtypes.bfloat16)
        h0_own = _jb_layout(h_0[:, bs, :]).reshape(L * 128, KT * BL).astype(np.float32)
        in_maps.append(
            {
                "xT": xT,
                "wi_sb": wi_sb,
                "wh_sb": wh_sb,
                "bias_sb": bias_sb,
                "bhn_sb": bhn_sb,
                "h0_own": np.ascontiguousarray(h0_own),
                }
        )

    nc = build_nc(S, has_bias)
    res = run_bass_kernel_spmd(nc, in_maps, core_ids=list(range(NC)))

    def unpack(a, lead):
        # [lead*128, KT*BL] -> [lead, BL, H]
        a = a.reshape(lead, 128, KT, BL)
        return a.transpose(0, 3, 2, 1).reshape(lead, BL, H)

    seq = np.concatenate(
        [unpack(r["out_seq"], S) for r in res.results], axis=1
    ).astype(np.float32)
    hn = np.concatenate(
        [unpack(r["out_hn"], L) for r in res.results], axis=1
    ).astype(np.float32)
    return seq, hn
